# revision 44
# baseline (speedup 1.0000x reference)
"""KimiMoEGate (sigmoid scoring, group-limited top-k) on 8 Trainium2 cores.

Strategy (hardcoded for hidden_states [4,4096,2048], weight [256,2048]):
  - Token-parallel: 16384 tokens sharded 2048/core across 8 cores; router
    weight + bias replicated per core.
  - Router logits at ~fp32 accuracy in 2.0 fp16-pass-equivalents on the PE:
      main:  xh(fp16) @ wh(fp16)          -> P0   (16 matmuls, 1 cyc/row)
      corrA: x8(fp8)  @ wl8(fp8)  2^-16   -> Pc   (8 DoubleRow matmuls)
      corrB: xl8(fp8) @ wh8(fp8)  2^-16   -> Pc   (8 DoubleRow matmuls)
    where xh=fp16(x), xl=x-xh, wh=fp16(1024W), wl=1024W-wh, and the fp8
    operands carry power-of-2 scales (x8: 2^5, wl8: 2^11, xl8: 2^16,
    wh8: 2^0) so both corrections land in one PSUM at scale 2^16.  The
    correction PSUM is folded into P0 by the PE itself via an identity
    matmul on an fp16 rescale of Pc, so sigmoid reads a single PSUM tile.
  - ACT applies sigmoid (scale 2^-10 folds away the 1024). Group-limited
    top-k via DVE max8 / max_index / match_replace on exact fp32 biased
    scores.
  - Bit-packing instead of a per-expert gather: the ranking tensor is
    rne(sfc*2^19)/2^19 + pbq[e]*2^-24 with pbq the expert bias quantized to
    4 bits. The top-8 values then carry their own bias: a +24/-24 magic
    round-trip splits them back into quantized score and packed bias, so
    w_j needs no gather.  The unpack + normalization runs once, batched
    over all tiles, at the end.
  - Token tiles are processed in PAIRS: two 128-token tiles share one
    2KB PSUM bank and one set of 512-wide elementwise ops, halving the
    per-instruction overhead of the ACT/Pool/DVE streams.
  - Four-stage software pipeline (sigmoid/sfc -> pack+group -> mask ->
    top-8) so no engine stream ever blocks at its head; gpsimd's
    cross-engine waits are SEQ-blocking, so its inputs must be old.
"""

import numpy as np
import ml_dtypes

from concourse import bacc, bass_utils
import concourse.mybir as mybir
from concourse.tile import TileContext

F16 = mybir.dt.float16
F32 = mybir.dt.float32
F8 = mybir.dt.float8e4
U16 = mybir.dt.uint16
U8 = mybir.dt.uint8
AF = mybir.ActivationFunctionType
ALU = mybir.AluOpType
AX = mybir.AxisListType
NPF8 = ml_dtypes.float8_e4m3

N_CORES = 8
N_GROUP = 8
EXP_PER_GROUP = 32
E = 256
H = 2048
H_CHUNKS = 16  # 2048 / 128
T_TOTAL = 16384
T_CORE = T_TOTAL // N_CORES
N_TILES = T_CORE // 128  # 16
N_WARM = 6

MAGIC = float(1.5 * 2.0 ** 23)  # integer-rounding magic at the 2^19 scale
QOFF = float(1.5 * 2.0 ** 4)    # same magic at the v0 scale: rounds to 2^-19

SX8 = 5    # x8  = fp8(x * 2^5)
SWL = 11   # wl8 = fp8(wl * 2^11)
SXL = 16   # xl8 = fp8(xl * 2^16)
SWH = 0    # wh8 = fp8(wh)
SCORR = float(2.0 ** -(SX8 + SWL))  # = 2^-16 = 2^-(SXL+SWH)


def build_kernel(nc, n_tiles=N_TILES):
    n_pairs = n_tiles // 2
    # per chunk 768 bytes: [0:256] xh (fp16), [256:384] x8 (fp8 of x*2^5),
    # [384:512] xl8 (fp8 of (x-xh)*2^16); one DMA per tile
    xall = nc.dram_tensor("xall", [n_tiles, 128, H_CHUNKS, 512], U8, kind="ExternalInput").ap()
    wh16 = nc.dram_tensor("wh16", [128, H_CHUNKS, E], F16, kind="ExternalInput").ap()
    # w88[:, ho, 0:256] = wl8 chunk (wl * 2^11), w88[:, ho, 256:512] = wh8 chunk
    w88 = nc.dram_tensor("w88", [128, H_CHUNKS, 2 * E], F8, kind="ExternalInput").ap()
    bias = nc.dram_tensor("bias_rep", [128, 2 * E], F32, kind="ExternalInput").ap()
    pbt = nc.dram_tensor("pb_rep", [128, 2 * E], F32, kind="ExternalInput").ap()
    rec = nc.dram_tensor("rec_const", [128, 2], F32, kind="ExternalInput").ap()
    ident = nc.dram_tensor("ident", [128, 128], F16, kind="ExternalInput").ap()
    idx_out = nc.dram_tensor("idx_out", [n_tiles, 128, 8], U16, kind="ExternalOutput").ap()
    wt_out = nc.dram_tensor("wt_out", [n_tiles, 128, 8], F32, kind="ExternalOutput").ap()

    with TileContext(nc) as tc:
        with (
            tc.tile_pool(name="const", bufs=1) as cpool,
            tc.tile_pool(name="xin", bufs=6) as xpool,
            tc.tile_pool(name="work", bufs=5) as wpool,
            tc.tile_pool(name="psum", bufs=4, space="PSUM") as ppool,
            tc.tile_pool(name="cpsum", bufs=3, space="PSUM") as cppool,
            tc.tile_pool(name="warmps", bufs=1, space="PSUM") as wppool,
            tc.tile_pool(name="persist", bufs=1) as perspool,
        ):
            # --- PE warm-up: tiny dummy matmuls with no DMA dependency;
            # they start the PE ramp clock before the first DMAs land.
            dummy = cpool.tile([128, 64], F16)
            nc.gpsimd.memset(dummy, 0.0)
            warm_ps = wppool.tile([128, 64], F32)
            for _ in range(N_WARM):
                nc.tensor.matmul(warm_ps[0:1, :], dummy[:, 0:1], dummy,
                                 start=True, stop=True)

            wh_sb = cpool.tile([128, H_CHUNKS, E], F16)
            w88_sb = cpool.tile([128, H_CHUNKS, 2 * E], F8)
            bias_in = cpool.tile([128, 2 * E], F32)
            bias_sb = cpool.tile([128, 2 * E], F32)
            pb_in = cpool.tile([128, 2 * E], F32)
            pb_sb = cpool.tile([128, 2 * E], F32)
            rec_in = cpool.tile([128, 2], F32)
            rec_sb = cpool.tile([128, 2], F32)
            ident_sb = cpool.tile([128, 128], F16)

            idx_u16 = perspool.tile([128, n_tiles, 8], U16)
            m8_all = perspool.tile([128, n_tiles, 8], F32)
            wt_all = perspool.tile([128, n_tiles, 8], F32)

            xtiles = {}

            def fetch(j):
                # fetch both tiles of pair j (one DMA per tile)
                if j >= n_pairs:
                    return
                for t in (2 * j, 2 * j + 1):
                    a = xpool.tile([128, H_CHUNKS, 512], U8, tag="xall")
                    nc.sync.dma_start(a, xall[t])
                    xtiles[t] = a

            # head order matches pair-0 consumption: x + fp8 weights first
            # (the correction matmuls run first), then fp16 main weights.
            for t in (0, 1):
                xall_head = xpool.tile([128, H_CHUNKS, 512], U8, tag="xall")
                xtiles[t] = xall_head
            for q in range(2):
                sl = slice(8 * q, 8 * q + 8)
                nc.sync.dma_start(xtiles[0][:, sl, :], xall[0][:, sl, :])
                nc.sync.dma_start(w88_sb[:, sl, :], w88[:, sl, :])
            for q in range(2):
                sl = slice(8 * q, 8 * q + 8)
                nc.sync.dma_start(wh_sb[:, sl, :], wh16[:, sl, :])
            nc.sync.dma_start(xtiles[1], xall[1])
            nc.sync.dma_start(ident_sb, ident)
            nc.sync.dma_start(bias_in, bias)
            nc.sync.dma_start(pb_in, pbt)
            nc.sync.dma_start(rec_in, rec)
            fetch(1)
            # engine-local copies so in-loop consumers depend on same-engine
            # producers (program order) instead of carrying DMA-sem waits.
            nc.gpsimd.tensor_copy(bias_sb, bias_in)
            nc.gpsimd.tensor_copy(pb_sb, pb_in)
            nc.vector.tensor_copy(rec_sb, rec_in)

            def stage_p(j, sfc):
                # packed ranking tensor: vq = rne(sfc*2^19)*2^-19 + pb
                # (scale/bias steps on ACT), paired 512-wide
                mt = wpool.tile([128, 2 * E], F32, tag="mt")
                nc.scalar.activation(mt, sfc, AF.Copy, bias=MAGIC,
                                     scale=float(2.0 ** 19))
                v0 = wpool.tile([128, 2 * E], F32, tag="v0")
                nc.scalar.activation(v0, mt, AF.Copy, bias=-QOFF,
                                     scale=float(2.0 ** -19))
                vq = wpool.tile([128, 2 * E], F32, tag="vq")
                nc.gpsimd.tensor_add(vq, v0, pb_sb)

                # group stage on exact fp32 sfc: top-2 per group of 32.
                # Reductions handle both tiles at once (16 groups); the
                # 8-wide ops (match_replace / max8 / threshold) are per tile.
                sfcg = sfc.rearrange("p (g e) -> p g e", g=2 * N_GROUP)
                g1 = wpool.tile([128, 2 * N_GROUP], F32, tag="g1")
                nc.vector.reduce_max(g1, sfcg, axis=AX.X)
                kn = wpool.tile([128, 2 * E], F32, tag="kn")
                nc.vector.match_replace(out=kn[:, :E], in_to_replace=g1[:, :8],
                                        in_values=sfc[:, :E], imm_value=-1e30)
                nc.vector.match_replace(out=kn[:, E:], in_to_replace=g1[:, 8:],
                                        in_values=sfc[:, E:], imm_value=-1e30)
                g2 = wpool.tile([128, 2 * N_GROUP], F32, tag="g2")
                nc.vector.reduce_max(g2, kn.rearrange("p (g e) -> p g e", g=2 * N_GROUP),
                                     axis=AX.X)
                gs = wpool.tile([128, 2 * N_GROUP], F32, tag="gs")
                nc.vector.tensor_add(gs, g1, g2)

                # top-4 groups per tile: threshold at 4th largest of 8
                g8a = wpool.tile([128, 8], F32, tag="g8a")
                nc.vector.max(out=g8a, in_=gs[:, :8])
                g8b = wpool.tile([128, 8], F32, tag="g8b")
                nc.vector.max(out=g8b, in_=gs[:, 8:])
                gm = wpool.tile([128, 2 * N_GROUP], F32, tag="gm")
                nc.vector.tensor_scalar(gm[:, :8], gs[:, :8], g8a[:, 3:4], None,
                                        op0=ALU.is_ge)
                nc.vector.tensor_scalar(gm[:, 8:], gs[:, 8:], g8b[:, 3:4], None,
                                        op0=ALU.is_ge)
                return vq, gm

            def stage_t(j, vq, gm):
                # mask the packed scores (Pool; its gm/vq deps are a full
                # iteration old, so the SEQ-blocking cross-engine wait that
                # gpsimd ops carry is already satisfied when reached)
                tmp = wpool.tile([128, 2 * N_GROUP, EXP_PER_GROUP], F32, tag="tmp")
                nc.gpsimd.tensor_mul(
                    tmp, vq.rearrange("p (g e) -> p g e", g=2 * N_GROUP),
                    gm.unsqueeze(2).to_broadcast([128, 2 * N_GROUP, EXP_PER_GROUP]))
                return tmp.rearrange("p g e -> p (g e)")

            def stage_m(j, tmpf):
                for t, sl in ((2 * j, slice(0, E)), (2 * j + 1, slice(E, 2 * E))):
                    m8 = wpool.tile([128, 8], F32, tag="m8")
                    nc.vector.max(out=m8, in_=tmpf[:, sl])
                    nc.vector.max_index(idx_u16[:, t, :], m8, tmpf[:, sl])
                    nc.vector.tensor_copy(m8_all[:, t, :], m8)

            def final_norm():
                # batched unpack + normalize for all tiles at once:
                # q8 = quantized sfc via the +24/-24 magic round-trip,
                # w = (pbv * rec0 + q8) - rec1, wt = w / sum(w) * 2.5
                m8f = m8_all.rearrange("p t k -> p (t k)")
                t1 = perspool.tile([128, n_tiles * 8], F32)
                nc.vector.tensor_scalar_add(t1, m8f, QOFF)
                q8 = perspool.tile([128, n_tiles * 8], F32)
                nc.vector.tensor_scalar_sub(q8, t1, QOFF)
                pbv = perspool.tile([128, n_tiles * 8], F32)
                nc.vector.tensor_sub(pbv, m8f, q8)
                u2 = perspool.tile([128, n_tiles * 8], F32)
                nc.vector.scalar_tensor_tensor(out=u2, in0=pbv,
                                               scalar=rec_sb[:, 0:1], in1=q8,
                                               op0=ALU.mult, op1=ALU.add)
                wr = perspool.tile([128, n_tiles, 8], F32)
                nc.vector.tensor_scalar(wr.rearrange("p t k -> p (t k)"), u2,
                                        rec_sb[:, 1:2], None, op0=ALU.subtract)
                s = perspool.tile([128, n_tiles], F32)
                nc.vector.reduce_sum(s, wr, axis=AX.X)
                r = perspool.tile([128, n_tiles], F32)
                nc.vector.reciprocal(r, s)
                nc.vector.scalar_tensor_tensor(
                    out=wt_all, in0=wr, scalar=2.5,
                    in1=r.unsqueeze(2).to_broadcast([128, n_tiles, 8]),
                    op0=ALU.mult, op1=ALU.mult)

            prev_a = None
            pend_t = []
            pend_m = []
            for j in range(n_pairs):
                fetch(j + 2)
                xta = xtiles.pop(2 * j)
                xtb = xtiles.pop(2 * j + 1)

                # paired PSUM tiles: [128, 2, 256] = one full 2KB bank
                pc = cppool.tile([128, 2, E], F32)
                p0 = ppool.tile([128, 2, E], F32)
                for half, xt in ((0, xta), (1, xtb)):
                    for hp in range(H_CHUNKS // 2):
                        sl = slice(2 * hp, 2 * hp + 2)
                        nc.tensor.matmul(pc[:, half, :],
                                         xt[:, sl, 256:384].bitcast(F8),
                                         w88_sb[:, sl, 0:E],
                                         start=(hp == 0), stop=False,
                                         perf_mode=mybir.MatmulPerfMode.DoubleRow)
                    for hp in range(H_CHUNKS // 2):
                        sl = slice(2 * hp, 2 * hp + 2)
                        nc.tensor.matmul(pc[:, half, :],
                                         xt[:, sl, 384:512].bitcast(F8),
                                         w88_sb[:, sl, E:],
                                         start=False,
                                         stop=(hp == H_CHUNKS // 2 - 1),
                                         perf_mode=mybir.MatmulPerfMode.DoubleRow)
                # u1 = Pc * 2^-16 in fp16 (one 512-wide ACT op)
                u1 = wpool.tile([128, 2, E], F16, tag="u1")
                nc.scalar.activation(u1, pc, AF.Copy, scale=SCORR)

                for half, xt in ((0, xta), (1, xtb)):
                    for ho in range(H_CHUNKS):
                        nc.tensor.matmul(p0[:, half, :],
                                         xt[:, ho, 0:256].bitcast(F16),
                                         wh_sb[:, ho, :],
                                         start=(ho == 0), stop=False)
                    # fold the correction in via an identity matmul
                    nc.tensor.matmul(p0[:, half, :], ident_sb, u1[:, half, :],
                                     start=False, stop=True)

                # four-stage software pipeline (see module docstring)
                if prev_a is not None:
                    vg = stage_p(*prev_a)
                    pend_t.append((prev_a[0], *vg))
                if len(pend_t) > 1:
                    jj, vqj, gmj = pend_t.pop(0)
                    pend_m.append((jj, stage_t(jj, vqj, gmj)))
                if len(pend_m) > 1:
                    stage_m(*pend_m.pop(0))

                scores = wpool.tile([128, 2 * E], F32, tag="scores")
                nc.scalar.activation(scores, p0.rearrange("p h e -> p (h e)"),
                                     AF.Sigmoid, scale=float(2.0 ** -10))
                sfc = wpool.tile([128, 2 * E], F32, tag="sfc")
                nc.gpsimd.tensor_add(sfc, scores, bias_sb)
                prev_a = (j, sfc)

                if j == n_pairs - 2:
                    # input prefetch is done; SP is free from here, so this
                    # wait blocks nothing.
                    nc.sync.dma_start(idx_out[:8].rearrange("t p k -> p t k"),
                                      idx_u16[:, :8, :])

            vg = stage_p(*prev_a)
            pend_t.append((prev_a[0], *vg))
            while pend_t:
                jj, vqj, gmj = pend_t.pop(0)
                pend_m.append((jj, stage_t(jj, vqj, gmj)))
                stage_m(*pend_m.pop(0))
            while pend_m:
                stage_m(*pend_m.pop(0))
            nc.sync.dma_start(idx_out[8:].rearrange("t p k -> p t k"),
                              idx_u16[:, 8:, :])
            final_norm()
            nc.scalar.dma_start(wt_out.rearrange("t p k -> p t k"), wt_all)
    return nc


def prep_core_inputs(x_core, shared):
    n_tiles = x_core.shape[0] // 128
    x = np.ascontiguousarray(x_core, dtype=np.float32)
    xh = x.astype(np.float16)
    xl = x - xh.astype(np.float32)
    x8 = np.clip(x * np.float32(2.0 ** SX8), -240, 240).astype(NPF8)
    xl8 = np.clip(xl * np.float32(2.0 ** SXL), -240, 240).astype(NPF8)

    def tile_x(a):
        # [T, H] -> [n_tiles, 128p(h_inner), 16(h_outer), 128(t)]
        return np.ascontiguousarray(
            a.reshape(n_tiles, 128, H_CHUNKS, 128).transpose(0, 3, 2, 1))

    xhb = tile_x(xh).view(np.uint8).reshape(n_tiles, 128, H_CHUNKS, 256)
    xall = np.concatenate(
        [xhb, tile_x(x8).view(np.uint8), tile_x(xl8).view(np.uint8)], axis=3)
    return {"xall": xall, **shared}


def prep_shared(weight, bias_vec):
    ws = np.ascontiguousarray(weight, dtype=np.float32) * 1024.0
    wh_ = ws.astype(np.float16)
    wl_ = ws - wh_.astype(np.float32)
    wl8 = np.clip(wl_ * np.float32(2.0 ** SWL), -240, 240).astype(NPF8)
    wh8 = np.clip(wh_.astype(np.float32) * np.float32(2.0 ** SWH),
                  -240, 240).astype(NPF8)

    def tile_w(a):
        # [E, H] -> [H, E] -> [128p(h_inner), 16(h_outer), E]
        return np.ascontiguousarray(a.T.reshape(H_CHUNKS, 128, E).transpose(1, 0, 2))

    w88 = np.concatenate([tile_w(wl8), tile_w(wh8)], axis=2)
    b = np.asarray(bias_vec, np.float32)
    bias_rep = np.broadcast_to(np.tile(b, 2), (128, 2 * E)).copy()

    # 4-bit packed bias: pbq in 0..15, quantum 2^-24 (stays below the 2^-19
    # ranking quantum so it never perturbs rank order beyond a tiebreak)
    bmin = np.float32(b.min())
    bmax = np.float32(b.max())
    step = np.float32((bmax - bmin) / 15.0) if bmax > bmin else np.float32(1.0)
    pbq = np.clip(np.round((b - bmin) / step), 0, 15).astype(np.float32)
    pb = (pbq * np.float32(2.0 ** -24)).astype(np.float32)
    pb_rep = np.broadcast_to(np.tile(pb, 2), (128, 2 * E)).copy()
    rec_const = np.broadcast_to(
        np.array([-np.float32(2.0 ** 24) * step, bmin], np.float32), (128, 2)).copy()
    return {"wh16": tile_w(wh_), "w88": w88, "bias_rep": bias_rep,
            "pb_rep": pb_rep, "rec_const": rec_const,
            "ident": np.eye(128, dtype=np.float16)}


_CACHED = {}


def _get_nc():
    if "nc" not in _CACHED:
        nc = bacc.Bacc("TRN2", num_devices=N_CORES)
        build_kernel(nc)
        nc.compile()
        _CACHED["nc"] = nc
    return _CACHED["nc"]


def make_in_maps(hidden_states, weight, e_score_correction_bias):
    x = np.asarray(hidden_states, np.float32).reshape(-1, H)
    shared = prep_shared(np.asarray(weight, np.float32),
                         np.asarray(e_score_correction_bias, np.float32))
    return [prep_core_inputs(x[c * T_CORE:(c + 1) * T_CORE], shared)
            for c in range(N_CORES)]


def kernel(hidden_states, weight, e_score_correction_bias):
    in_maps = make_in_maps(hidden_states, weight, e_score_correction_bias)
    nc = _get_nc()
    res = bass_utils.run_bass_kernel_spmd(nc, in_maps, core_ids=list(range(N_CORES)))
    idx = np.concatenate([r["idx_out"].reshape(-1, 8) for r in res.results], axis=0)
    wt = np.concatenate([r["wt_out"].reshape(-1, 8) for r in res.results], axis=0)
    return idx.astype(np.int32), wt.astype(np.float32)


# revision 45
# speedup vs baseline: 1.0318x; 1.0318x over previous
"""KimiMoEGate (sigmoid scoring, group-limited top-k) on 8 Trainium2 cores.

Strategy (hardcoded for hidden_states [4,4096,2048], weight [256,2048]):
  - Token-parallel: 16384 tokens sharded 2048/core across 8 cores; router
    weight + bias replicated per core.
  - Router logits at ~fp32 accuracy in 2.0 fp16-pass-equivalents on the PE:
      main:  xh(fp16) @ wh(fp16)          -> P0   (16 matmuls, 1 cyc/row)
      corrA: x8(fp8)  @ wl8(fp8)  2^-16   -> Pc   (8 DoubleRow matmuls)
      corrB: xl8(fp8) @ wh8(fp8)  2^-16   -> Pc   (8 DoubleRow matmuls)
    where xh=fp16(x), xl=x-xh, wh=fp16(1024W), wl=1024W-wh, and the fp8
    operands carry power-of-2 scales (x8: 2^5, wl8: 2^11, xl8: 2^16,
    wh8: 2^0) so both corrections land in one PSUM at scale 2^16.  The
    correction PSUM is folded into P0 by the PE itself via an identity
    matmul on an fp16 rescale of Pc, so sigmoid reads a single PSUM tile.
  - ACT applies sigmoid (scale 2^-10 folds away the 1024). Group-limited
    top-k via DVE max8 / max_index / match_replace on exact fp32 biased
    scores.
  - Bit-packing instead of a per-expert gather: the ranking tensor is
    rne(sfc*2^19)/2^19 + pbq[e]*2^-24 with pbq the expert bias quantized to
    4 bits. The top-8 values then carry their own bias: a +24/-24 magic
    round-trip splits them back into quantized score and packed bias, so
    w_j needs no gather.  The unpack + normalization runs once, batched
    over all tiles, at the end.
  - Token tiles are processed in PAIRS: two 128-token tiles share one
    2KB PSUM bank and one set of 512-wide elementwise ops, halving the
    per-instruction overhead of the ACT/Pool/DVE streams.
  - Four-stage software pipeline (sigmoid/sfc -> pack+group -> mask ->
    top-8) so no engine stream ever blocks at its head; gpsimd's
    cross-engine waits are SEQ-blocking, so its inputs must be old.
"""

import numpy as np
import ml_dtypes

from concourse import bacc, bass_utils
import concourse.mybir as mybir
from concourse.tile import TileContext

F16 = mybir.dt.float16
F32 = mybir.dt.float32
F8 = mybir.dt.float8e4
U16 = mybir.dt.uint16
U8 = mybir.dt.uint8
AF = mybir.ActivationFunctionType
ALU = mybir.AluOpType
AX = mybir.AxisListType
NPF8 = ml_dtypes.float8_e4m3

N_CORES = 8
N_GROUP = 8
EXP_PER_GROUP = 32
E = 256
H = 2048
H_CHUNKS = 16  # 2048 / 128
T_TOTAL = 16384
T_CORE = T_TOTAL // N_CORES
N_TILES = T_CORE // 128  # 16
N_WARM = 6

MAGIC = float(1.5 * 2.0 ** 23)  # integer-rounding magic at the 2^19 scale
QOFF = float(1.5 * 2.0 ** 4)    # same magic at the v0 scale: rounds to 2^-19

SX8 = 5    # x8  = fp8(x * 2^5)
SWL = 11   # wl8 = fp8(wl * 2^11)
SXL = 16   # xl8 = fp8(xl * 2^16)
SWH = 0    # wh8 = fp8(wh)
SCORR = float(2.0 ** -(SX8 + SWL))  # = 2^-16 = 2^-(SXL+SWH)


def build_kernel(nc, n_tiles=N_TILES):
    n_pairs = n_tiles // 2
    # per chunk 768 bytes: [0:256] xh (fp16), [256:384] x8 (fp8 of x*2^5),
    # [384:512] xl8 (fp8 of (x-xh)*2^16); one DMA per tile
    xall = nc.dram_tensor("xall", [n_tiles, 128, H_CHUNKS, 512], U8, kind="ExternalInput").ap()
    wh16 = nc.dram_tensor("wh16", [128, H_CHUNKS, E], F16, kind="ExternalInput").ap()
    # w88[:, ho, 0:256] = wl8 chunk (wl * 2^11), w88[:, ho, 256:512] = wh8 chunk
    w88 = nc.dram_tensor("w88", [128, H_CHUNKS, 2 * E], F8, kind="ExternalInput").ap()
    bias = nc.dram_tensor("bias_rep", [128, 2 * E], F32, kind="ExternalInput").ap()
    pbt = nc.dram_tensor("pb_rep", [128, 2 * E], F32, kind="ExternalInput").ap()
    rec = nc.dram_tensor("rec_const", [128, 2], F32, kind="ExternalInput").ap()
    ident = nc.dram_tensor("ident", [128, 128], F16, kind="ExternalInput").ap()
    idx_out = nc.dram_tensor("idx_out", [n_tiles, 128, 8], U16, kind="ExternalOutput").ap()
    wt_out = nc.dram_tensor("wt_out", [n_tiles, 128, 8], F32, kind="ExternalOutput").ap()

    with TileContext(nc) as tc:
        with (
            tc.tile_pool(name="const", bufs=1) as cpool,
            tc.tile_pool(name="xin", bufs=6) as xpool,
            tc.tile_pool(name="work", bufs=5) as wpool,
            tc.tile_pool(name="psum", bufs=4, space="PSUM") as ppool,
            tc.tile_pool(name="cpsum", bufs=3, space="PSUM") as cppool,
            tc.tile_pool(name="warmps", bufs=1, space="PSUM") as wppool,
            tc.tile_pool(name="persist", bufs=1) as perspool,
        ):
            # --- PE warm-up: tiny dummy matmuls with no DMA dependency;
            # they start the PE ramp clock before the first DMAs land.
            dummy = cpool.tile([128, 64], F16)
            nc.gpsimd.memset(dummy, 0.0)
            warm_ps = wppool.tile([128, 64], F32)
            for _ in range(N_WARM):
                nc.tensor.matmul(warm_ps[0:1, :], dummy[:, 0:1], dummy,
                                 start=True, stop=True)

            wh_sb = cpool.tile([128, H_CHUNKS, E], F16)
            w88_sb = cpool.tile([128, H_CHUNKS, 2 * E], F8)
            bias_in = cpool.tile([128, 2 * E], F32)
            bias_sb = cpool.tile([128, 2 * E], F32)
            pb_in = cpool.tile([128, 2 * E], F32)
            pb_sb = cpool.tile([128, 2 * E], F32)
            rec_in = cpool.tile([128, 2], F32)
            rec_sb = cpool.tile([128, 2], F32)
            ident_sb = cpool.tile([128, 128], F16)

            idx_u16 = perspool.tile([128, n_tiles, 8], U16)
            m8_all = perspool.tile([128, n_tiles, 8], F32)
            wt_all = perspool.tile([128, n_tiles, 8], F32)

            xtiles = {}

            def fetch(j):
                # fetch both tiles of pair j (one DMA per tile)
                if j >= n_pairs:
                    return
                for t in (2 * j, 2 * j + 1):
                    a = xpool.tile([128, H_CHUNKS, 512], U8, tag="xall")
                    nc.sync.dma_start(a, xall[t])
                    xtiles[t] = a

            # head order matches pair-0 consumption: x + fp8 weights first
            # (the correction matmuls run first), then fp16 main weights.
            for t in (0, 1):
                xall_head = xpool.tile([128, H_CHUNKS, 512], U8, tag="xall")
                xtiles[t] = xall_head
            for q in range(2):
                sl = slice(8 * q, 8 * q + 8)
                nc.sync.dma_start(xtiles[0][:, sl, :], xall[0][:, sl, :])
                nc.sync.dma_start(w88_sb[:, sl, :], w88[:, sl, :])
            for q in range(2):
                sl = slice(8 * q, 8 * q + 8)
                nc.sync.dma_start(wh_sb[:, sl, :], wh16[:, sl, :])
            nc.sync.dma_start(xtiles[1], xall[1])
            nc.sync.dma_start(ident_sb, ident)
            nc.sync.dma_start(bias_in, bias)
            nc.sync.dma_start(pb_in, pbt)
            nc.sync.dma_start(rec_in, rec)
            fetch(1)
            # engine-local copies so in-loop consumers depend on same-engine
            # producers (program order) instead of carrying DMA-sem waits.
            nc.gpsimd.tensor_copy(bias_sb, bias_in)
            nc.gpsimd.tensor_copy(pb_sb, pb_in)
            nc.vector.tensor_copy(rec_sb, rec_in)

            def stage_p(j, sfc):
                # packed ranking tensor: vq = rne(sfc*2^19)*2^-19 + pb
                # (scale/bias steps on ACT), paired 512-wide
                mt = wpool.tile([128, 2 * E], F32, tag="mt")
                nc.scalar.activation(mt, sfc, AF.Copy, bias=MAGIC,
                                     scale=float(2.0 ** 19))
                v0 = wpool.tile([128, 2 * E], F32, tag="v0")
                nc.scalar.activation(v0, mt, AF.Copy, bias=-QOFF,
                                     scale=float(2.0 ** -19))
                vq = wpool.tile([128, 2 * E], F32, tag="vq")
                nc.gpsimd.tensor_add(vq, v0, pb_sb)

                # group stage on exact fp32 sfc: top-2 per group of 32.
                # Reductions handle both tiles at once (16 groups); the
                # 8-wide ops (match_replace / max8 / threshold) are per tile.
                sfcg = sfc.rearrange("p (g e) -> p g e", g=2 * N_GROUP)
                g1 = wpool.tile([128, 2 * N_GROUP], F32, tag="g1")
                nc.vector.reduce_max(g1, sfcg, axis=AX.X)
                kn = wpool.tile([128, 2 * E], F32, tag="kn")
                nc.vector.match_replace(out=kn[:, :E], in_to_replace=g1[:, :8],
                                        in_values=sfc[:, :E], imm_value=-1e30)
                nc.vector.match_replace(out=kn[:, E:], in_to_replace=g1[:, 8:],
                                        in_values=sfc[:, E:], imm_value=-1e30)
                g2 = wpool.tile([128, 2 * N_GROUP], F32, tag="g2")
                nc.vector.reduce_max(g2, kn.rearrange("p (g e) -> p g e", g=2 * N_GROUP),
                                     axis=AX.X)
                gs = wpool.tile([128, 2 * N_GROUP], F32, tag="gs")
                nc.vector.tensor_add(gs, g1, g2)

                # top-4 groups per tile: threshold at 4th largest of 8
                g8a = wpool.tile([128, 8], F32, tag="g8a")
                nc.vector.max(out=g8a, in_=gs[:, :8])
                g8b = wpool.tile([128, 8], F32, tag="g8b")
                nc.vector.max(out=g8b, in_=gs[:, 8:])
                gm = wpool.tile([128, 2 * N_GROUP], F32, tag="gm")
                nc.vector.tensor_scalar(gm[:, :8], gs[:, :8], g8a[:, 3:4], None,
                                        op0=ALU.is_ge)
                nc.vector.tensor_scalar(gm[:, 8:], gs[:, 8:], g8b[:, 3:4], None,
                                        op0=ALU.is_ge)
                return vq, gm

            def stage_t(j, vq, gm):
                # mask the packed scores (Pool; its gm/vq deps are a full
                # iteration old, so the SEQ-blocking cross-engine wait that
                # gpsimd ops carry is already satisfied when reached)
                tmp = wpool.tile([128, 2 * N_GROUP, EXP_PER_GROUP], F32, tag="tmp")
                nc.gpsimd.tensor_mul(
                    tmp, vq.rearrange("p (g e) -> p g e", g=2 * N_GROUP),
                    gm.unsqueeze(2).to_broadcast([128, 2 * N_GROUP, EXP_PER_GROUP]))
                return tmp.rearrange("p g e -> p (g e)")

            def stage_m(j, tmpf):
                for t, sl in ((2 * j, slice(0, E)), (2 * j + 1, slice(E, 2 * E))):
                    m8 = wpool.tile([128, 8], F32, tag="m8")
                    nc.vector.max(out=m8, in_=tmpf[:, sl])
                    nc.vector.max_index(idx_u16[:, t, :], m8, tmpf[:, sl])
                    nc.vector.tensor_copy(m8_all[:, t, :], m8)

            def final_norm():
                # batched unpack + normalize for all tiles at once:
                # q8 = quantized sfc via the +24/-24 magic round-trip,
                # w = (pbv * rec0 + q8) - rec1, wt = w / sum(w) * 2.5
                m8f = m8_all.rearrange("p t k -> p (t k)")
                t1 = perspool.tile([128, n_tiles * 8], F32)
                nc.vector.tensor_scalar_add(t1, m8f, QOFF)
                q8 = perspool.tile([128, n_tiles * 8], F32)
                nc.vector.tensor_scalar_sub(q8, t1, QOFF)
                pbv = perspool.tile([128, n_tiles * 8], F32)
                nc.vector.tensor_sub(pbv, m8f, q8)
                u2 = perspool.tile([128, n_tiles * 8], F32)
                nc.vector.scalar_tensor_tensor(out=u2, in0=pbv,
                                               scalar=rec_sb[:, 0:1], in1=q8,
                                               op0=ALU.mult, op1=ALU.add)
                wr = perspool.tile([128, n_tiles, 8], F32)
                nc.vector.tensor_scalar(wr.rearrange("p t k -> p (t k)"), u2,
                                        rec_sb[:, 1:2], None, op0=ALU.subtract)
                s = perspool.tile([128, n_tiles], F32)
                nc.vector.reduce_sum(s, wr, axis=AX.X)
                r = perspool.tile([128, n_tiles], F32)
                nc.vector.reciprocal(r, s)
                nc.vector.scalar_tensor_tensor(
                    out=wt_all, in0=wr, scalar=2.5,
                    in1=r.unsqueeze(2).to_broadcast([128, n_tiles, 8]),
                    op0=ALU.mult, op1=ALU.mult)

            prev_a = None
            pend_t = []
            pend_m = []
            for j in range(n_pairs):
                fetch(j + 2)
                xta = xtiles.pop(2 * j)
                xtb = xtiles.pop(2 * j + 1)

                # paired PSUM tiles: [128, 2, 256] = one full 2KB bank
                pc = cppool.tile([128, 2, E], F32)
                p0 = ppool.tile([128, 2, E], F32)
                u1 = wpool.tile([128, 2, E], F16, tag="u1")
                for half, xt in ((0, xta), (1, xtb)):
                    for hp in range(H_CHUNKS // 2):
                        sl = slice(2 * hp, 2 * hp + 2)
                        nc.tensor.matmul(pc[:, half, :],
                                         xt[:, sl, 256:384].bitcast(F8),
                                         w88_sb[:, sl, 0:E],
                                         start=(hp == 0), stop=False,
                                         perf_mode=mybir.MatmulPerfMode.DoubleRow)
                    for hp in range(H_CHUNKS // 2):
                        sl = slice(2 * hp, 2 * hp + 2)
                        nc.tensor.matmul(pc[:, half, :],
                                         xt[:, sl, 384:512].bitcast(F8),
                                         w88_sb[:, sl, E:],
                                         start=False,
                                         stop=(hp == H_CHUNKS // 2 - 1),
                                         perf_mode=mybir.MatmulPerfMode.DoubleRow)
                    # u1 = Pc * 2^-16 in fp16, per half, so each identity
                    # matmul's input is ready well before the PE needs it
                    nc.scalar.activation(u1[:, half, :], pc[:, half, :],
                                         AF.Copy, scale=SCORR)

                for half, xt in ((0, xta), (1, xtb)):
                    for ho in range(H_CHUNKS):
                        nc.tensor.matmul(p0[:, half, :],
                                         xt[:, ho, 0:256].bitcast(F16),
                                         wh_sb[:, ho, :],
                                         start=(ho == 0), stop=False)
                    # fold the correction in via an identity matmul
                    nc.tensor.matmul(p0[:, half, :], ident_sb, u1[:, half, :],
                                     start=False, stop=True)

                # four-stage software pipeline (see module docstring)
                if prev_a is not None:
                    vg = stage_p(*prev_a)
                    pend_t.append((prev_a[0], *vg))
                if len(pend_t) > 1:
                    jj, vqj, gmj = pend_t.pop(0)
                    pend_m.append((jj, stage_t(jj, vqj, gmj)))
                if len(pend_m) > 1:
                    stage_m(*pend_m.pop(0))

                scores = wpool.tile([128, 2 * E], F32, tag="scores")
                nc.scalar.activation(scores, p0.rearrange("p h e -> p (h e)"),
                                     AF.Sigmoid, scale=float(2.0 ** -10))
                sfc = wpool.tile([128, 2 * E], F32, tag="sfc")
                nc.gpsimd.tensor_add(sfc, scores, bias_sb)
                prev_a = (j, sfc)

                if j == n_pairs - 2:
                    # input prefetch is done; SP is free from here, so this
                    # wait blocks nothing.
                    nc.sync.dma_start(idx_out[:8].rearrange("t p k -> p t k"),
                                      idx_u16[:, :8, :])

            vg = stage_p(*prev_a)
            pend_t.append((prev_a[0], *vg))
            while pend_t:
                jj, vqj, gmj = pend_t.pop(0)
                pend_m.append((jj, stage_t(jj, vqj, gmj)))
                stage_m(*pend_m.pop(0))
            while pend_m:
                stage_m(*pend_m.pop(0))
            nc.sync.dma_start(idx_out[8:].rearrange("t p k -> p t k"),
                              idx_u16[:, 8:, :])
            final_norm()
            nc.scalar.dma_start(wt_out.rearrange("t p k -> p t k"), wt_all)
    return nc


def prep_core_inputs(x_core, shared):
    n_tiles = x_core.shape[0] // 128
    x = np.ascontiguousarray(x_core, dtype=np.float32)
    xh = x.astype(np.float16)
    xl = x - xh.astype(np.float32)
    x8 = np.clip(x * np.float32(2.0 ** SX8), -240, 240).astype(NPF8)
    xl8 = np.clip(xl * np.float32(2.0 ** SXL), -240, 240).astype(NPF8)

    def tile_x(a):
        # [T, H] -> [n_tiles, 128p(h_inner), 16(h_outer), 128(t)]
        return np.ascontiguousarray(
            a.reshape(n_tiles, 128, H_CHUNKS, 128).transpose(0, 3, 2, 1))

    xhb = tile_x(xh).view(np.uint8).reshape(n_tiles, 128, H_CHUNKS, 256)
    xall = np.concatenate(
        [xhb, tile_x(x8).view(np.uint8), tile_x(xl8).view(np.uint8)], axis=3)
    return {"xall": xall, **shared}


def prep_shared(weight, bias_vec):
    ws = np.ascontiguousarray(weight, dtype=np.float32) * 1024.0
    wh_ = ws.astype(np.float16)
    wl_ = ws - wh_.astype(np.float32)
    wl8 = np.clip(wl_ * np.float32(2.0 ** SWL), -240, 240).astype(NPF8)
    wh8 = np.clip(wh_.astype(np.float32) * np.float32(2.0 ** SWH),
                  -240, 240).astype(NPF8)

    def tile_w(a):
        # [E, H] -> [H, E] -> [128p(h_inner), 16(h_outer), E]
        return np.ascontiguousarray(a.T.reshape(H_CHUNKS, 128, E).transpose(1, 0, 2))

    w88 = np.concatenate([tile_w(wl8), tile_w(wh8)], axis=2)
    b = np.asarray(bias_vec, np.float32)
    bias_rep = np.broadcast_to(np.tile(b, 2), (128, 2 * E)).copy()

    # 4-bit packed bias: pbq in 0..15, quantum 2^-24 (stays below the 2^-19
    # ranking quantum so it never perturbs rank order beyond a tiebreak)
    bmin = np.float32(b.min())
    bmax = np.float32(b.max())
    step = np.float32((bmax - bmin) / 15.0) if bmax > bmin else np.float32(1.0)
    pbq = np.clip(np.round((b - bmin) / step), 0, 15).astype(np.float32)
    pb = (pbq * np.float32(2.0 ** -24)).astype(np.float32)
    pb_rep = np.broadcast_to(np.tile(pb, 2), (128, 2 * E)).copy()
    rec_const = np.broadcast_to(
        np.array([-np.float32(2.0 ** 24) * step, bmin], np.float32), (128, 2)).copy()
    return {"wh16": tile_w(wh_), "w88": w88, "bias_rep": bias_rep,
            "pb_rep": pb_rep, "rec_const": rec_const,
            "ident": np.eye(128, dtype=np.float16)}


_CACHED = {}


def _get_nc():
    if "nc" not in _CACHED:
        nc = bacc.Bacc("TRN2", num_devices=N_CORES)
        build_kernel(nc)
        nc.compile()
        _CACHED["nc"] = nc
    return _CACHED["nc"]


def make_in_maps(hidden_states, weight, e_score_correction_bias):
    x = np.asarray(hidden_states, np.float32).reshape(-1, H)
    shared = prep_shared(np.asarray(weight, np.float32),
                         np.asarray(e_score_correction_bias, np.float32))
    return [prep_core_inputs(x[c * T_CORE:(c + 1) * T_CORE], shared)
            for c in range(N_CORES)]


def kernel(hidden_states, weight, e_score_correction_bias):
    in_maps = make_in_maps(hidden_states, weight, e_score_correction_bias)
    nc = _get_nc()
    res = bass_utils.run_bass_kernel_spmd(nc, in_maps, core_ids=list(range(N_CORES)))
    idx = np.concatenate([r["idx_out"].reshape(-1, 8) for r in res.results], axis=0)
    wt = np.concatenate([r["wt_out"].reshape(-1, 8) for r in res.results], axis=0)
    return idx.astype(np.int32), wt.astype(np.float32)
